# revision 9
# baseline (speedup 1.0000x reference)
"""Trainium2 Bass kernel for the 16-head MHA problem (B=4, S=2048, D=1024).

Key identity: the reference ADDS mask*2^32 (positive!) to the raw scores.
In fp32, every masked score collapses to exactly 2^32 (|score| << 256 makes
the rounding exact), so after the 1/8 scale and softmax every row with at
least one masked entry becomes exactly  indicator / row_count  -- the SAME
probability matrix P for every head and every batch (Q and K are never
needed).  The MHA therefore collapses end-to-end:

    out[b] = P @ values[b] @ (Wv @ Wo) + (bv @ Wo + bo)

with P = triu(1, k=1)/row_count.  W = Wv @ Wo (1024x1024) is precomputed on
the host; the only remaining device work per core is ONE dense GEMM plus a
causal suffix-average with trivial structure.

Sharding: 8 cores = 4 batches x 2 output-column halves (512 wide each).
Per core, computed in TRANSPOSED orientation with the sequence REVERSED:

    A_rev^T [512, 2048] = W_half^T @ values[b]^T[:, ::-1]     (PE, fp16)
    scan    = per-chunk prefix sums along seq, seeded with the
              running chunk offset (Vector tensor_tensor_scan;
              reversed seq = suffix sums)
    out^T   = scan * (1/count)                                (GpSimd mul)

Chunk totals fall out of the PSUM->SBUF eviction for free (scalar engine
activation accum_out), so the chunk scans are independent of each other;
the final seq chunk is only 128 wide to keep the post-GEMM tail short.
Row q of the output is column t = 2046-q; the single row with no masked
entries (q = S-1) gets a true softmax, patched on the host from the raw
inputs via reassociation.

The data path runs in fp16 (full PE rate; ~1e-3 end-to-end L2 error).
"""

import numpy as np

import concourse.bass as bass
import concourse.mybir as mybir
import concourse.tile as tile
from concourse import bacc, bass_utils

# ---------------------------------------------------------------- constants
B, S, D = 4, 2048, 1024
HEADS, DK = 16, 64
NH = 2                      # output-column halves
HWID = D // NH              # 512 output columns per core
N_CORES = B * NH            # 8
NKT = D // 128              # 8 contraction k-tiles
NOC = HWID // 128           # 4 outcol tiles per core
CS = [512, 512, 512, 384, 128]          # seq chunk sizes (short tail chunk)
GOFF = [0, 512, 1024, 1536, 1920]
NCH = len(CS)
MASK_CONST = np.float32(4294967296.0)   # +2^32, faithful to the reference
SCALE = 1.0 / np.sqrt(np.float32(DK))   # 1/8

F32 = mybir.dt.float32
FP16 = mybir.dt.float16
BF16 = mybir.dt.bfloat16
ADD = mybir.AluOpType.add
BYPASS = mybir.AluOpType.bypass
COPY = mybir.ActivationFunctionType.Copy


# ------------------------------------------------------------- kernel build
def _build():
    nc = bacc.Bacc("TRN2", target_bir_lowering=False, debug=False,
                   num_devices=N_CORES)

    def din(name, shape, dt):
        return nc.dram_tensor(name, shape, dt, kind="ExternalInput").ap()

    # weights interleaved with the first seq chunk so the k-pair DMA pieces
    # feed the chunk-0 GEMM in consumption order
    wx = din("wx", (128, NKT, 1024), FP16)     # [:,k,0:512]=W_k  [:,k,512:]=chunk0
    xrc = [din(f"xr{c}", (128, NKT, CS[c]), FP16) for c in range(1, NCH)]
    recip = din("recip", (128, S), FP16)       # 1/(t+1) broadcast rows

    out = nc.dram_tensor("out", (NOC, 128, S), FP16, kind="ExternalOutput").ap()
    warm_out = nc.dram_tensor("warm_out", (128, 128), F32,
                              kind="ExternalOutput").ap()

    with tile.TileContext(nc) as tc:
        with (
            tc.tile_pool(name="res", bufs=1) as res,
            tc.tile_pool(name="small", bufs=1) as small,
            tc.tile_pool(name="outp", bufs=4) as outp,
            tc.tile_pool(name="ppsum", bufs=2, space="PSUM") as ppsum,
        ):
            wx_sb = res.tile([128, NKT, 1024], FP16, tag="wx")
            xr_sb = [res.tile([128, NKT, CS[c]], FP16, tag=f"xr{c}",
                              name=f"xr{c}_sb")
                     for c in range(1, NCH)]
            recip_sb = res.tile([128, S], FP16, tag="recip")
            a_sb = res.tile([128, NOC, S], FP16, tag="a")
            scan_sb = res.tile([128, NOC, S], FP16, tag="scan")
            tot_sb = small.tile([128, NOC, NCH], F32, tag="tot")
            off_sb = small.tile([128, NOC, NCH], F32, tag="off")
            scr = small.tile([128, 128], BF16, tag="scr")
            warm_sb = small.tile([128, 128], F32, tag="warm")

            nc.vector.memset(scr[:], 1.0)
            nc.vector.memset(off_sb[:], 0.0)

            # PE warm-up while the first DMAs land (HAM to K=8/8)
            wmp = ppsum.tile([128, NOC, 512], F32, tag="ps")
            for _ in range(20):
                nc.tensor.matmul(wmp[:, 0, 0:128], scr[:], scr[:],
                                 start=True, stop=True)
            nc.scalar.copy(warm_sb[:], wmp[:, 0, 0:128])
            nc.scalar.dma_start(warm_out[:], warm_sb[:])

            # ------------- input DMAs, in exact consumption order
            for kk in range(4):
                nc.sync.dma_start(wx_sb[:, 2 * kk:2 * kk + 2, :],
                                  wx[:, 2 * kk:2 * kk + 2, :])
            nc.sync.dma_start(xr_sb[0][:], xrc[0][:])
            nc.sync.dma_start(xr_sb[1][:], xrc[1][:])
            nc.sync.dma_start(recip_sb[:], recip[:])
            nc.sync.dma_start(xr_sb[2][:], xrc[2][:])
            nc.sync.dma_start(xr_sb[3][:], xrc[3][:])

            # ------------- main pipeline over seq chunks
            for c in range(NCH):
                cs, go = CS[c], GOFF[c]
                ps = ppsum.tile([128, NOC, 512], F32, tag="ps")
                for k in range(NKT):
                    if c == 0:
                        rhs = wx_sb[:, k, 512:512 + cs]
                    else:
                        rhs = xr_sb[c - 1][:, k, :]
                    for oc in range(NOC):
                        nc.tensor.matmul(
                            ps[:, oc, 0:cs],
                            wx_sb[:, k, oc * 128:(oc + 1) * 128],
                            rhs, start=(k == 0), stop=(k == NKT - 1))
                for oc in range(NOC):
                    # eviction + free running chunk total for the scan seed
                    nc.scalar.activation(
                        a_sb[:, oc, go:go + cs], ps[:, oc, 0:cs], COPY,
                        accum_out=tot_sb[:, oc, c:c + 1])
                    if c + 1 < NCH:
                        nc.vector.tensor_add(
                            off_sb[:, oc, c + 1:c + 2],
                            off_sb[:, oc, c:c + 1],
                            tot_sb[:, oc, c:c + 1])
                    nc.vector.tensor_tensor_scan(
                        scan_sb[:, oc, go:go + cs],
                        a_sb[:, oc, go:go + cs], a_sb[:, oc, go:go + cs],
                        initial=off_sb[:, oc, c:c + 1],
                        op0=ADD, op1=BYPASS)
                    ot = outp.tile([128, 512], FP16, tag="ot")
                    nc.gpsimd.tensor_mul(
                        ot[:, 0:cs], scan_sb[:, oc, go:go + cs],
                        recip_sb[:, go:go + cs])
                    nc.sync.dma_start(out[oc, :, go:go + cs], ot[:, 0:cs])

    nc.compile()
    return nc


# ------------------------------------------------------------- host wrapper
_CACHE: dict = {}
LAST_RESULTS = None
LAST_IN_MAPS = None


def _get_kernel():
    if "v4" not in _CACHE:
        _CACHE["v4"] = _build()
    return _CACHE["v4"]


def _patch_rows(out, qfix, queries, keys, values, mask2d,
                Wq, bq_, Wk, bk_, Wv, bv_, Wo, bo_):
    """True softmax for rows with no masked entry, via reassociation so the
    big Q/K projections are never materialized (pure fp32 numpy)."""
    q = qfix
    nq = len(q)
    mrow = mask2d[q] * MASK_CONST                       # [nq, S]
    for b in range(B):
        Qr = queries[b][q] @ Wq + bq_                   # [nq, HEADS*DK]
        Oc = np.empty((nq, HEADS * DK), dtype=np.float32)
        for H in range(HEADS):
            hs = slice(H * DK, (H + 1) * DK)
            t = Qr[:, hs] @ Wk[:, hs].T                 # [nq, D]
            sc = t @ keys[b].T                          # [nq, S]
            sc = sc + (Qr[:, hs] @ bk_[hs])[:, None]    # K-bias term
            y = (sc + mrow) * np.float32(SCALE)
            y = y - y.max(axis=1, keepdims=True)
            e = np.exp(y, dtype=np.float32)
            p = (e / e.sum(axis=1, keepdims=True)).astype(np.float32)
            z = p @ values[b]                           # [nq, D]
            Oc[:, hs] = z @ Wv[:, hs] + bv_[hs]
        out[b][q] = Oc @ Wo + bo_


def _host_fallback(queries, keys, values, mask2d,
                   Wq, bq_, Wk, bk_, Wv, bv_, Wo, bo_):
    """Exact numpy mirror of the reference; only used if the mask is not the
    expected causal-complement pattern."""
    out = np.empty((B, S, D), dtype=np.float32)
    madd = mask2d * MASK_CONST
    for b in range(B):
        Q = queries[b] @ Wq + bq_
        K = keys[b] @ Wk + bk_
        V = values[b] @ Wv + bv_
        O = np.empty((S, HEADS * DK), dtype=np.float32)
        for H in range(HEADS):
            hs = slice(H * DK, (H + 1) * DK)
            scv = (Q[:, hs] @ K[:, hs].T + madd) * np.float32(SCALE)
            scv = scv - scv.max(axis=1, keepdims=True)
            e = np.exp(scv, dtype=np.float32)
            p = e / e.sum(axis=1, keepdims=True)
            O[:, hs] = p @ V[:, hs]
        out[b] = O @ Wo + bo_
    return out


def kernel(queries, keys, values, mask, Wq, bq, Wk, bk, Wv, bv, Wo, bo):
    queries = np.asarray(queries, dtype=np.float32)
    keys = np.asarray(keys, dtype=np.float32)
    values = np.asarray(values, dtype=np.float32)
    mask2d = np.ascontiguousarray(
        np.asarray(mask, dtype=np.float32).reshape(S, S))
    Wq = np.asarray(Wq, dtype=np.float32); bq_ = np.asarray(bq, dtype=np.float32)
    Wk = np.asarray(Wk, dtype=np.float32); bk_ = np.asarray(bk, dtype=np.float32)
    Wv = np.asarray(Wv, dtype=np.float32); bv_ = np.asarray(bv, dtype=np.float32)
    Wo = np.asarray(Wo, dtype=np.float32); bo_ = np.asarray(bo, dtype=np.float32)

    # Rows whose masked entries collapse to the row max (reference fp32
    # semantics).  The kernel hardcodes the causal-complement structure;
    # verify it and fall back to exact host compute otherwise.
    ind = ((mask2d * MASK_CONST) == MASK_CONST)
    if not np.array_equal(ind, np.triu(np.ones((S, S), dtype=bool), k=1)) or \
            not np.all((mask2d == 0.0) | (mask2d == 1.0)):
        return _host_fallback(queries, keys, values, mask2d,
                              Wq, bq_, Wk, bk_, Wv, bv_, Wo, bo_)
    qfix = np.array([S - 1])

    nc = _get_kernel()

    W = (Wv @ Wo).astype(np.float32)                    # [1024, 1024]
    rowbias = bv_ @ Wo + bo_                            # [1024]

    # 1/(t+1) rows, broadcast to all 128 partitions
    rrow = (1.0 / np.arange(1, S + 1, dtype=np.float32)).astype(np.float16)
    recip_np = np.ascontiguousarray(np.broadcast_to(rrow, (128, S)))

    # reversed-seq V^T: vr8[k, p, t] = values[b][S-1-t, k*128+p]
    wxs, xrs = {}, {}
    for b in range(B):
        vr8 = values[b].T[:, ::-1].astype(np.float16).reshape(NKT, 128, S)
        wxs[b] = np.ascontiguousarray(vr8[:, :, 0:512].transpose(1, 0, 2))
        xrs[b] = {
            f"xr{c}": np.ascontiguousarray(
                vr8[:, :, GOFF[c]:GOFF[c] + CS[c]].transpose(1, 0, 2))
            for c in range(1, NCH)
        }

    in_maps = []
    wp = {}
    for j in range(NH):
        Wh = W[:, j * HWID:(j + 1) * HWID].astype(np.float16)
        wp[j] = Wh.reshape(NKT, 128, HWID).transpose(1, 0, 2)
    for core in range(N_CORES):
        b, j = divmod(core, NH)
        wx_np = np.ascontiguousarray(
            np.concatenate([wp[j], wxs[b]], axis=2))
        im = {"wx": wx_np, "recip": recip_np}
        im.update(xrs[b])
        in_maps.append(im)

    res = bass_utils.run_bass_kernel_spmd(
        nc, in_maps, core_ids=list(range(N_CORES)))

    global LAST_RESULTS, LAST_IN_MAPS
    LAST_RESULTS = res
    LAST_IN_MAPS = in_maps

    out = np.empty((B, S, D), dtype=np.float32)
    for core in range(N_CORES):
        b, j = divmod(core, NH)
        outT = res.results[core]["out"].reshape(HWID, S).astype(np.float32)
        out[b][0:S - 1, j * HWID:(j + 1) * HWID] = outT[:, 0:S - 1][:, ::-1].T

    if np.any(rowbias):
        out += rowbias

    _patch_rows(out, qfix, queries, keys, values, mask2d,
                Wq, bq_, Wk, bk_, Wv, bv_, Wo, bo_)
    return out
